# revision 9
# baseline (speedup 1.0000x reference)
"""Additive (Bahdanau) attention kernel for Trainium2, 8 NeuronCores.

Problem shapes (hardcoded): B=8, TQ=128, TV=256, D=512, U=256.
Sharding: data-parallel over batch B -> one batch element per core.

v2: ACT-bound design. The span floor is the tanh on the scalar engine
(TQ*TV*U = 8.4M elems/core at 1 elem/cyc/lane @ 1.2 GHz ~= 56us), so
every other engine is kept strictly below that and ACT does nothing but
big-block tanh + the 4 softmax exps:
  - all matmul operands bf16 (1 cyc/col streaming vs ~2 for f32r/fp32
    measured on HW); V window single bf16 (no hi/lo split) -> score
    matmuls halve twice vs v1 (128 MMs x ~216ns).
  - pre-adds on DVE as bf16 tensor_scalar (2x_1P mode, ~196ns/[128,256]
    vs 344ns fp32).
  - projections evacuate psum via DVE (not ACT).
  - softmax: the window matmul puts even-q score in cols 0:256 and odd-q
    in cols 256:512 of EVERY row of the pair, so exp is done per parity
    (accum_out rowsums), scaled by 1/rowsum BEFORE transpose (per-
    partition scalar), then parity-split transposes; the ctx matmul
    reads even/odd q columns via stride-2 lhsT APs. No predicated
    selects, no post-scale on psum.
  - a dummy tanh right after the first small DMA pulls the ~2.7us ACT
    table load into the DMA window.
"""
import sys
import numpy as np

if '/opt/trn_rl_repo' not in sys.path:
    sys.path.insert(0, '/opt/trn_rl_repo')

import ml_dtypes

BF = ml_dtypes.bfloat16

B, TQ, TV, D, U = 8, 128, 256, 512, 256
P = 128          # partitions
KD = D // P      # 4 k-chunks over d
CU = U // P      # 2 chunks over u
CV = TV // P     # 2 chunks over v
# q-block sizes: short head (fast pipeline fill), a block boundary at
# q=64 (half A softmax), short tail (fast drain)
BLOCKS = [8, 16, 16, 16, 8, 16, 16, 16, 8, 8]
assert sum(BLOCKS) == TQ
assert 64 in np.cumsum(BLOCKS)

_compiled = None


def _build():
    import concourse.bass as bass
    import concourse.tile as tile
    from concourse import bacc, mybir

    f32 = mybir.dt.float32
    bf16 = mybir.dt.bfloat16
    AF = mybir.ActivationFunctionType

    nc = bacc.Bacc("TRN2", target_bir_lowering=False, debug=False,
                   enable_asserts=True, num_devices=B)

    W1_d = nc.dram_tensor("W1B", [P, KD, U], bf16, kind="ExternalInput").ap()
    W2_d = nc.dram_tensor("W2B", [P, KD, U], bf16, kind="ExternalInput").ap()
    QT_d = nc.dram_tensor("QTB", [P, KD, TQ], bf16, kind="ExternalInput").ap()
    VT_d = nc.dram_tensor("VTB", [P, KD, TV], bf16, kind="ExternalInput").ap()
    VAL_d = nc.dram_tensor("VALB", [P, CV, D], bf16, kind="ExternalInput").ap()
    VW_d = nc.dram_tensor("VWB", [P, CU, 256], bf16, kind="ExternalInput").ap()
    B12_d = nc.dram_tensor("B12", [P, CU], f32, kind="ExternalInput").ap()
    ID_d = nc.dram_tensor("IDB", [P, P], bf16, kind="ExternalInput").ap()
    # [q//2, q%2, d] view of the [TQ, D] output: even/odd q rows are
    # addressable without step slicing
    OUT_d = nc.dram_tensor("OUT", [TQ // 2, 2, D], f32,
                           kind="ExternalOutput").ap()

    with tile.TileContext(nc) as tc:
        with (
            tc.tile_pool(name="cst", bufs=1) as cst,
            tc.tile_pool(name="pre_p", bufs=2) as pre_p,
            tc.tile_pool(name="feat_p", bufs=2) as feat_p,
            tc.tile_pool(name="sm", bufs=1) as sm,
            tc.tile_pool(name="ps", bufs=1, space=bass.MemorySpace.PSUM) as ps,
        ):
            # ---- inputs; chunk the projection operands so matmuls can
            # start before the full tensors arrive ----
            b12 = cst.tile([P, CU], f32, tag="b12")
            nc.gpsimd.dma_start(b12[:], B12_d)
            idt = cst.tile([P, P], bf16, tag="idt")
            nc.gpsimd.dma_start(idt[:], ID_d)
            # dummy tanh: pulls the ACT table load into the DMA window
            warm = sm.tile([P, 2], f32, tag="warm")
            nc.scalar.activation(warm[:], b12[:], AF.Tanh)

            w1 = cst.tile([P, KD, U], bf16, tag="w1")
            vt = cst.tile([P, KD, TV], bf16, tag="vt")
            w2 = cst.tile([P, KD, U], bf16, tag="w2")
            qt = cst.tile([P, KD, TQ], bf16, tag="qt")
            # spread the projection-critical transfers across engine DMA
            # queues so they run in parallel (one queue is ~85 GB/s)
            engs = [nc.sync, nc.scalar, nc.gpsimd, nc.sync]
            for k in range(KD):
                engs[k].dma_start(w2[:, k, :], W2_d[:, k, :])
                engs[k].dma_start(qt[:, k, :], QT_d[:, k, :])
            for k in range(KD):
                engs[k].dma_start(w1[:, k, :], W1_d[:, k, :])
                engs[k].dma_start(vt[:, k, :], VT_d[:, k, :])
            valb = cst.tile([P, CV, D], bf16, tag="valb")
            nc.sync.dma_start(valb[:], VAL_d)
            vwb = cst.tile([P, CU, 256], bf16, tag="vwb")
            nc.gpsimd.dma_start(vwb[:], VW_d)

            # ---- projections (PE bf16, psum fp32) ----
            psW1 = ps.tile([P, CU, TV], f32, tag="psW1")   # one bank
            psW2 = ps.tile([P, CU, TQ], f32, tag="psW2")   # half bank
            w1vT = cst.tile([P, CU, TV], bf16, tag="w1vT")
            w2qT = cst.tile([P, CU, TQ], f32, tag="w2qT")  # TS scalar: f32

            def project_w2(qh):
                qs = slice(qh * 64, qh * 64 + 64)
                for c in range(CU):
                    for k in range(KD):
                        nc.tensor.matmul(psW2[:, c, qs],
                                         w2[:, k, c * P:(c + 1) * P],
                                         qt[:, k, qs],
                                         start=(k == 0), stop=(k == KD - 1))
                # bias b1+b2 folded into w2q (pre = w1v + w2q + b1 + b2)
                for c in range(CU):
                    nc.vector.tensor_scalar_add(w2qT[:, c, qs],
                                                psW2[:, c, qs],
                                                b12[:, c:c + 1])

            def project_w1(vh):
                vs = slice(vh * P, vh * P + P)
                for c in range(CU):
                    for k in range(KD):
                        nc.tensor.matmul(psW1[:, c, vs],
                                         w1[:, k, c * P:(c + 1) * P],
                                         vt[:, k, vs],
                                         start=(k == 0), stop=(k == KD - 1))
                nc.vector.tensor_copy(w1vT[:, :, vs], psW1[:, :, vs])

            project_w2(0)
            project_w1(0)
            project_w1(1)

            # ---- score phase (two psum groups: q<64 and q>=64) ----
            score_A = ps.tile([P, 2 * TV], f32, tag="scoreA")  # one bank
            score_B = ps.tile([P, 2 * TV], f32, tag="scoreB")  # one bank
            att_e = sm.tile([P, TV], bf16, tag="att_e")
            att_o = sm.tile([P, TV], bf16, tag="att_o")
            att_es = sm.tile([P, TV], bf16, tag="att_es")
            att_os = sm.tile([P, TV], bf16, tag="att_os")
            sums = sm.tile([P, 4], f32, tag="sums")
            psT_e = ps.tile([P, CV, 64], bf16, tag="psTe")
            psT_o = ps.tile([P, CV, 64], bf16, tag="psTo")
            attnT_e = sm.tile([P, CV, 64], bf16, tag="attnTe")
            attnT_o = sm.tile([P, CV, 64], bf16, tag="attnTo")
            ctx_ps = ps.tile([P, D], f32, tag="ctx")           # one bank
            ctx = sm.tile([P, D], f32, tag="ctxsb")

            def softmax_context(half):
                """Softmax + transpose + context matmul for one q-half.

                score rows: every partition q of the pair holds the even
                q's score in cols 0:TV and the odd q's in TV:2TV, so the
                e/o tiles are valid on even/odd partitions respectively
                (other partitions hold the partner's finite score).
                """
                h0 = half * 64
                score_ps = score_A if half == 0 else score_B
                nc.scalar.activation(att_e[:], score_ps[:, 0:TV], AF.Exp,
                                     accum_out=sums[:, 0:1])
                nc.scalar.activation(att_o[:], score_ps[:, TV:2 * TV], AF.Exp,
                                     accum_out=sums[:, 1:2])
                # normalize before transpose (per-partition scalar works
                # here; after transpose the axis is wrong for TS)
                nc.vector.reciprocal(sums[:, 2:3], sums[:, 0:1])
                nc.vector.reciprocal(sums[:, 3:4], sums[:, 1:2])
                nc.vector.tensor_scalar_mul(att_es[:], att_e[:], sums[:, 2:3])
                nc.vector.tensor_scalar_mul(att_os[:], att_o[:], sums[:, 3:4])
                for c in range(CV):
                    nc.tensor.transpose(psT_e[:, c, :],
                                        att_es[h0:h0 + 64, c * P:(c + 1) * P],
                                        idt[h0:h0 + 64, h0:h0 + 64])
                    nc.tensor.transpose(psT_o[:, c, :],
                                        att_os[h0:h0 + 64, c * P:(c + 1) * P],
                                        idt[h0:h0 + 64, h0:h0 + 64])
                nc.vector.tensor_copy(attnT_e[:], psT_e[:])
                nc.vector.tensor_copy(attnT_o[:], psT_o[:])
                # ctx rows 0:32 = even q (h0,h0+2,..), 32:64 = odd
                for c in range(CV):
                    nc.tensor.matmul(ctx_ps[0:32, :],
                                     attnT_e[:, c, 0:64:2],
                                     valb[:, c, :],
                                     start=(c == 0), stop=(c == CV - 1),
                                     tile_position=(0, 0))
                for c in range(CV):
                    nc.tensor.matmul(ctx_ps[32:64, :],
                                     attnT_o[:, c, 1:64:2],
                                     valb[:, c, :],
                                     start=(c == 0), stop=(c == CV - 1),
                                     tile_position=(0, 32))
                nc.vector.tensor_copy(ctx[0:64, :], ctx_ps[0:64, :])
                p0 = h0 // 2
                nc.sync.dma_start(OUT_d[p0:p0 + 32, 0, :], ctx[0:32, :])
                nc.sync.dma_start(OUT_d[p0:p0 + 32, 1, :], ctx[32:64, :])

            mmA = [0, (TQ // 4) * CU]   # counter, total for half A
            mmB = [0, (TQ // 4) * CU]
            q0 = 0
            for bi, bq in enumerate(BLOCKS):
                pre = pre_p.tile([P, CU, 16, TV], bf16, tag="pre")
                feat = feat_p.tile([P, CU, 16, TV], bf16, tag="feat")
                if bi == 0:
                    # v-split: start adding as soon as half of w1vT is ready
                    for vh in range(2):
                        vs = slice(vh * P, vh * P + P)
                        for ql in range(bq):
                            q = q0 + ql
                            for c in range(CU):
                                nc.vector.tensor_scalar_add(
                                    pre[:, c, ql, vs], w1vT[:, c, vs],
                                    w2qT[:, c, q:q + 1])
                        for c in range(CU):
                            nc.scalar.activation(feat[:, c, 0:bq, vs],
                                                 pre[:, c, 0:bq, vs],
                                                 AF.Tanh)
                else:
                    for ql in range(bq):
                        q = q0 + ql
                        for c in range(CU):
                            nc.vector.tensor_scalar_add(
                                pre[:, c, ql, :], w1vT[:, c, :],
                                w2qT[:, c, q:q + 1])
                    nc.scalar.activation(feat[:, :, 0:bq, :],
                                         pre[:, :, 0:bq, :], AF.Tanh)
                for pl in range(bq // 2):
                    q = q0 + 2 * pl
                    score_ps, mmc = (score_A, mmA) if q < 64 else (score_B, mmB)
                    for c in range(CU):
                        rhs = feat[:, c, 2 * pl:2 * pl + 2, :]
                        nc.tensor.matmul(score_ps[:],
                                         vwb[:, c, 127 - q:255 - q],
                                         rhs,
                                         start=(mmc[0] == 0),
                                         stop=(mmc[0] == mmc[1] - 1))
                        mmc[0] += 1
                q0 += bq
                if bi == 0:
                    project_w2(1)
                if q0 == 64:
                    softmax_context(0)
            softmax_context(1)

    nc.compile()
    return nc


def _prep_shared(W1, b1, W2, b2, V, bv):
    Vf = np.asarray(V, np.float32)[:, 0]
    Vw = np.zeros((P, CU, 256), np.float32)
    for c in range(CU):
        Vw[:, c, 127] = Vf[c * P:(c + 1) * P]
        Vw[:, c, 128] = Vf[c * P:(c + 1) * P]
    b12 = (b1 + b2).astype(np.float32).reshape(CU, P).T.copy()
    ident = np.eye(P, dtype=np.float32)
    return {
        "VWB": Vw.astype(BF),
        "B12": np.ascontiguousarray(b12),
        "IDB": ident.astype(BF),
    }


def kernel(query, values, W1, b1, W2, b2, V, bv, _trace=False, _tmpdir=None):
    global _compiled
    from concourse.bass_utils import run_bass_kernel_spmd

    query = np.asarray(query, np.float32)
    values = np.asarray(values, np.float32)
    shared = _prep_shared(np.asarray(W1), np.asarray(b1), np.asarray(W2),
                          np.asarray(b2), np.asarray(V), np.asarray(bv))
    W1c = np.ascontiguousarray(
        np.asarray(W1, np.float32).reshape(KD, P, U).transpose(1, 0, 2))
    W2c = np.ascontiguousarray(
        np.asarray(W2, np.float32).reshape(KD, P, U).transpose(1, 0, 2))
    shared["W1B"] = W1c.astype(BF)
    shared["W2B"] = W2c.astype(BF)

    if _compiled is None:
        _compiled = _build()
    nc = _compiled

    in_maps = []
    for i in range(B):
        m = dict(shared)
        qT = query[i].T.reshape(KD, P, TQ).transpose(1, 0, 2)
        vT = values[i].T.reshape(KD, P, TV).transpose(1, 0, 2)
        vl = values[i].reshape(CV, P, D).transpose(1, 0, 2)
        m["QTB"] = np.ascontiguousarray(qT).astype(BF)
        m["VTB"] = np.ascontiguousarray(vT).astype(BF)
        m["VALB"] = np.ascontiguousarray(vl).astype(BF)
        in_maps.append(m)

    kw = {}
    if _trace:
        kw.update(trace=True, tmpdir=_tmpdir)
    res = run_bass_kernel_spmd(nc, in_maps, core_ids=list(range(B)), **kw)
    out = np.stack([res.results[i]["OUT"].reshape(TQ, D) for i in range(B)],
                   axis=0)
    if _trace:
        kernel._last_trace = res
    return out
